# revision 17
# baseline (speedup 1.0000x reference)
"""VQ codebook eval forward on 8 Trainium2 NeuronCores.

Data-parallel over tokens: z [4, 256, 16, 32, 32] -> 65536 tokens of dim 256,
8192 tokens per core.  Codebook [2048, 256] replicated.

Per core (Bass/Tile kernel):
  - scores[t, c] = z_t . e_c - ||e_c||^2/2  computed on the PE with tokens on
    PSUM partitions and codes on the free dim.  The matmul runs in fp16
    hi/lo split form (z = zh + zl, e = e1 + e2; zh.e1 + zh.e2 + zl.e1) which
    carries ~fp32 precision at 3x the fp32 matmul rate; the per-code bias
    -||e||^2/2 is folded in as a 2-row fp16 matmul against a ones vector.
  - argmax over codes via DVE max8/find_index8 -> encoding index + best score
  - emb_q rows gathered from the DRAM codebook via indirect DMA, transposed
    back to channel-major with PE transpose, DMA'd out
  - sum(z^2) per chunk via ScalarE Square+accum (for the commitment loss)
Host: assembles shards and computes the scalar outputs (loss / perplexity /
usage) from the tiny per-core staging tensors.
"""

import os
import sys
from contextlib import ExitStack

import numpy as np

sys.path.insert(0, "/opt/trn_rl_repo")

import concourse.bacc as bacc
import concourse.bass as bass
import concourse.tile as tile
from concourse import mybir
from concourse.bass import IndirectOffsetOnAxis
from concourse.bass_utils import run_bass_kernel_spmd

F32 = mybir.dt.float32
F16 = mybir.dt.float16
U32 = mybir.dt.uint32

N_CORES = 8
B, D, T, H, W = 4, 256, 16, 32, 32
N_CODES = 2048
N_TOK = B * T * H * W            # 65536
S = N_TOK // N_CORES             # 8192 tokens per core
CHUNK = 512                      # tokens per load chunk
NCHUNK = S // CHUNK              # 16
MT = CHUNK // 128                # 4 token-tiles (128) per chunk
KT = D // 128                    # 2 contraction tiles
NC_CHUNK = 512                   # code chunk per matmul (psum bank)

MODE = os.environ.get("VQ_MODE", "f16x3")


def build_nc(nchunk=NCHUNK, mode=MODE):
    """Build the per-core Bass program. nchunk can be reduced for sim tests."""
    s = nchunk * CHUNK
    nmt = nchunk * MT
    nc = bacc.Bacc("TRN2", target_bir_lowering=False, debug=False)

    zt = nc.dram_tensor("zt", [KT, 128, s], F32, kind="ExternalInput")
    emb = nc.dram_tensor("emb", [N_CODES, D], F32, kind="ExternalInput")
    ident = nc.dram_tensor("ident", [128, 128], F32, kind="ExternalInput")
    if mode == "fp32":
        embT = nc.dram_tensor("embT", [KT, 128, N_CODES], F32,
                              kind="ExternalInput")
        biasr = nc.dram_tensor("biasr", [1, N_CODES], F32,
                               kind="ExternalInput")
    else:
        eh1 = nc.dram_tensor("eh1", [KT, 128, N_CODES], F16,
                             kind="ExternalInput")
        eh2 = nc.dram_tensor("eh2", [KT, 128, N_CODES], F16,
                             kind="ExternalInput")
        bias2 = nc.dram_tensor("bias2", [2, N_CODES], F16,
                               kind="ExternalInput")

    eq = nc.dram_tensor("eq", [KT, 128, s], F32, kind="ExternalOutput")
    idx_o = nc.dram_tensor("idx", [128, nmt], U32, kind="ExternalOutput")
    maxv_o = nc.dram_tensor("maxv", [128, nmt], F32, kind="ExternalOutput")
    sumsq_o = nc.dram_tensor("sumsq", [128, nchunk], F32,
                             kind="ExternalOutput")

    with tile.TileContext(nc) as tc, ExitStack() as ctx:
        consts = ctx.enter_context(tc.tile_pool(name="consts", bufs=1))
        zpool = ctx.enter_context(tc.tile_pool(name="z", bufs=3))
        zsplit = ctx.enter_context(tc.tile_pool(name="zs", bufs=3))
        scpool = ctx.enter_context(tc.tile_pool(name="scores", bufs=3))
        sqpool = ctx.enter_context(tc.tile_pool(name="sq", bufs=2))
        gpool = ctx.enter_context(tc.tile_pool(name="gather", bufs=3))
        opool = ctx.enter_context(tc.tile_pool(name="out", bufs=3))
        stage = ctx.enter_context(tc.tile_pool(name="stage", bufs=1))
        pspool = ctx.enter_context(tc.tile_pool(name="ps", bufs=3,
                                                space="PSUM"))
        tppool = ctx.enter_context(tc.tile_pool(name="tp", bufs=2,
                                                space="PSUM"))

        # --- constants resident in SBUF ---
        ident_sb = consts.tile([128, 128], F32)
        nc.sync.dma_start(out=ident_sb, in_=ident[:, :])
        if mode == "fp32":
            e_sb = consts.tile([128, KT, N_CODES], F32)
            for k in range(KT):
                nc.sync.dma_start(out=e_sb[:, k, :], in_=embT[k, :, :])
            biasr_sb = consts.tile([1, N_CODES], F32)
            nc.sync.dma_start(out=biasr_sb, in_=biasr[:, :])
            ones_sb = consts.tile([1, 128], F32)
            nc.vector.memset(ones_sb, 1.0)
        else:
            e1_sb = consts.tile([128, KT, N_CODES], F16)
            e2_sb = consts.tile([128, KT, N_CODES], F16)
            for k in range(KT):
                nc.sync.dma_start(out=e1_sb[:, k, :], in_=eh1[k, :, :])
                nc.sync.dma_start(out=e2_sb[:, k, :], in_=eh2[k, :, :])
            bias2_sb = consts.tile([2, N_CODES], F16)
            nc.sync.dma_start(out=bias2_sb, in_=bias2[:, :])
            ones2_sb = consts.tile([2, 128], F16)
            nc.vector.memset(ones2_sb, 1.0)

        # --- staging for small outputs ---
        idxstage = stage.tile([128, nmt, 8], U32)
        maxstage = stage.tile([128, nmt, 8], F32)
        sumsqstage = stage.tile([128, nchunk], F32)

        # PE warmup: dense dummy matmuls chew through the HAM cold window
        # while the big const/z DMAs land.
        warm = consts.tile([128, 128], F16)
        nc.vector.memset(warm, 0.0)
        wps = tppool.tile([128, 128], F32, tag="tp")
        for _ in range(96):
            nc.tensor.matmul(wps, lhsT=warm, rhs=warm, start=True, stop=True)

        for c in range(nchunk):
            t0 = c * CHUNK
            zc = zpool.tile([128, KT, CHUNK], F32)
            for k in range(KT):
                nc.sync.dma_start(out=zc[:, k, :], in_=zt[k, :, t0:t0 + CHUNK])

            # sum of squares of this chunk (both k-tiles at once)
            sq = sqpool.tile([128, KT, CHUNK], F32)
            nc.scalar.activation(
                out=sq, in_=zc,
                func=mybir.ActivationFunctionType.Square,
                accum_out=sumsqstage[:, c:c + 1],
            )

            if mode != "fp32":
                zh = zsplit.tile([128, KT, CHUNK], F16)
                nc.scalar.activation(
                    out=zh, in_=zc, func=mybir.ActivationFunctionType.Copy)
                zl = zsplit.tile([128, KT, CHUNK], F16)
                nc.gpsimd.tensor_tensor(
                    out=zl, in0=zc, in1=zh, op=mybir.AluOpType.subtract)

            for m in range(MT):
                mt = c * MT + m
                tl = m * 128

                sc = scpool.tile([128, N_CODES], F32)
                for half in range(2):
                    ps = pspool.tile([128, 2 * NC_CHUNK], F32)
                    for nl in range(2):
                        n = half * 2 + nl
                        pcols = slice(nl * NC_CHUNK, (nl + 1) * NC_CHUNK)
                        gcols = slice(n * NC_CHUNK, (n + 1) * NC_CHUNK)
                        if mode == "fp32":
                            for k in range(KT):
                                nc.tensor.matmul(
                                    ps[:, pcols],
                                    lhsT=zc[:, k, tl:tl + 128],
                                    rhs=e_sb[:, k, gcols],
                                    start=(k == 0), stop=False,
                                )
                            nc.tensor.matmul(
                                ps[:, pcols],
                                lhsT=ones_sb[:, :],
                                rhs=biasr_sb[:, gcols],
                                start=False, stop=True,
                            )
                        else:
                            first = True
                            for (zp, ep) in ((zh, e1_sb), (zl, e1_sb),
                                             (zh, e2_sb)):
                                for k in range(KT):
                                    nc.tensor.matmul(
                                        ps[:, pcols],
                                        lhsT=zp[:, k, tl:tl + 128],
                                        rhs=ep[:, k, gcols],
                                        start=first, stop=False,
                                    )
                                    first = False
                            nc.tensor.matmul(
                                ps[:, pcols],
                                lhsT=ones2_sb[:, :],
                                rhs=bias2_sb[:, gcols],
                                start=False, stop=True,
                            )
                    # PSUM -> SBUF so DVE max runs at SBUF speed, PSUM frees
                    nc.scalar.activation(
                        out=sc[:, half * 1024:(half + 1) * 1024], in_=ps,
                        func=mybir.ActivationFunctionType.Copy,
                    )

                nc.vector.max(out=maxstage[:, mt, :], in_=sc)
                nc.vector.max_index(
                    out=idxstage[:, mt, :],
                    in_max=maxstage[:, mt, :],
                    in_values=sc,
                )

                # gather emb rows for the 128 tokens of this tile
                eq_tok = gpool.tile([128, D], F32)
                nc.gpsimd.indirect_dma_start(
                    out=eq_tok,
                    out_offset=None,
                    in_=emb[:, :],
                    in_offset=IndirectOffsetOnAxis(
                        ap=idxstage[:, mt, 0:1], axis=0),
                )
                # transpose to channel-major and write out
                eqT = opool.tile([128, KT, 128], F32)
                for g in range(KT):
                    tp = tppool.tile([128, 128], F32)
                    nc.tensor.transpose(
                        tp, eq_tok[:, g * 128:(g + 1) * 128], ident_sb)
                    nc.scalar.activation(
                        out=eqT[:, g, :], in_=tp,
                        func=mybir.ActivationFunctionType.Copy,
                    )
                    nc.sync.dma_start(
                        out=eq[g, :, t0 + tl:t0 + tl + 128], in_=eqT[:, g, :])

                # flush idx/maxv stats every 16 token-tiles so the kernel
                # tail isn't serialized on one big compaction
                if mt % 16 == 15 or mt == nmt - 1:
                    f0 = (mt // 16) * 16
                    fw = mt - f0 + 1
                    idxf = opool.tile([128, 16], U32)
                    nc.vector.tensor_copy(
                        idxf[:, :fw], idxstage[:, f0:f0 + fw, 0])
                    maxf = opool.tile([128, 16], F32)
                    nc.vector.tensor_copy(
                        maxf[:, :fw], maxstage[:, f0:f0 + fw, 0])
                    nc.sync.dma_start(out=idx_o[:, f0:f0 + fw],
                                      in_=idxf[:, :fw])
                    nc.sync.dma_start(out=maxv_o[:, f0:f0 + fw],
                                      in_=maxf[:, :fw])

        nc.sync.dma_start(out=sumsq_o[:, :], in_=sumsqstage)

    nc.compile()
    return nc


_NC_CACHE = {}


def _get_nc():
    key = (NCHUNK, MODE)
    if key not in _NC_CACHE:
        _NC_CACHE[key] = build_nc()
    return _NC_CACHE[key]


def make_inputs(z, embeddings, mode=MODE):
    """Host-side prep: per-core input maps."""
    z = np.ascontiguousarray(np.asarray(z, dtype=np.float32))
    emb = np.ascontiguousarray(np.asarray(embeddings, dtype=np.float32))
    # [b, ktile, 128, s_total]
    zr = z.reshape(B, KT, 128, T * H * W)
    ident = np.eye(128, dtype=np.float32)
    common = {"emb": emb, "ident": ident}
    bias = (-0.5 * (emb.astype(np.float64) ** 2).sum(axis=1))
    if mode == "fp32":
        common["embT"] = np.ascontiguousarray(emb.T).reshape(KT, 128, N_CODES)
        common["biasr"] = bias.astype(np.float32)[None, :]
    else:
        embT = np.ascontiguousarray(emb.T).astype(np.float32)
        e1 = embT.astype(np.float16)
        e2 = (embT - e1.astype(np.float32)).astype(np.float16)
        common["eh1"] = e1.reshape(KT, 128, N_CODES)
        common["eh2"] = e2.reshape(KT, 128, N_CODES)
        bias32 = bias.astype(np.float32)
        b1 = bias32.astype(np.float16)
        b2 = (bias32 - b1.astype(np.float32)).astype(np.float16)
        common["bias2"] = np.stack([b1, b2])
    in_maps = []
    per_batch = T * H * W            # 16384
    cores_per_batch = per_batch // S  # 2
    for i in range(N_CORES):
        b = i // cores_per_batch
        off = (i % cores_per_batch) * S
        zt_i = np.ascontiguousarray(zr[b, :, :, off:off + S])
        in_maps.append({"zt": zt_i, **common})
    return in_maps


def postprocess(results, z=None):
    """Assemble full outputs from per-core result dicts."""
    eq_full = np.empty((B, D, T * H * W), dtype=np.float32)
    idx_full = np.empty((N_TOK,), dtype=np.int64)
    maxv_sum = 0.0
    sumsq_sum = 0.0
    per_batch = T * H * W
    cores_per_batch = per_batch // S
    for i, r in enumerate(results):
        b = i // cores_per_batch
        off = (i % cores_per_batch) * S
        eq_full[b, :, off:off + S] = r["eq"].reshape(D, S)
        # idx [128, nmt]: token (mt*128 + p) -> [p, mt]
        idx_full[i * S:(i + 1) * S] = r["idx"].T.reshape(S).astype(np.int64)
        maxv_sum += r["maxv"].astype(np.float64).sum()
        sumsq_sum += r["sumsq"].astype(np.float64).sum()

    embeddings_st = eq_full.reshape(B, D, T, H, W)
    encoding_indices = idx_full.reshape(B, T, H, W).astype(np.int32)

    # commitment loss. Device-side identity: sum (z - e*)^2 =
    # sum ||z||^2 - 2 * sum max_score (f64-accurate). To match the
    # reference's f32 arithmetic bit-for-bit, recompute the mean with the
    # same eager jax-CPU op sequence the reference uses when z is available.
    commitment_loss = None
    if z is not None and os.environ.get("VQ_FAST_LOSS", "0") != "1":
        try:
            import jax
            import jax.numpy as jnp
            with jax.default_device(jax.devices("cpu")[0]):
                zj = jnp.asarray(np.asarray(z, dtype=np.float32))
                qj = jnp.asarray(embeddings_st)
                cl = jnp.mean((zj - qj) ** 2)
                cl = jnp.minimum(cl, 10.0) * 0.25
                commitment_loss = np.float32(cl)
        except Exception:
            commitment_loss = None
    if commitment_loss is None:
        total = sumsq_sum - 2.0 * maxv_sum
        mean = np.float32(total / (N_TOK * D))
        commitment_loss = np.float32(min(float(mean), 10.0) * 0.25)

    counts = np.bincount(idx_full, minlength=N_CODES).astype(np.float32)
    avg_probs = counts / np.float32(N_TOK)
    perplexity = np.float32(
        np.exp(-np.sum(avg_probs * np.log(avg_probs + np.float32(1e-7)))))
    num_codes_used = np.int32((avg_probs > 0).sum())
    code_usage_ratio = np.float32(num_codes_used / np.float32(N_CODES))

    return (embeddings_st, encoding_indices, commitment_loss,
            perplexity, num_codes_used, code_usage_ratio)


def kernel(z, embeddings):
    nc = _get_nc()
    in_maps = make_inputs(z, embeddings)
    res = run_bass_kernel_spmd(nc, in_maps, core_ids=list(range(N_CORES)))
    return postprocess(res.results, z=z)


# revision 19
# speedup vs baseline: 1.0037x; 1.0037x over previous
"""VQ codebook eval forward on 8 Trainium2 NeuronCores.

Data-parallel over tokens: z [4, 256, 16, 32, 32] -> 65536 tokens of dim 256,
8192 tokens per core.  Codebook [2048, 256] replicated.

Per core (Bass/Tile kernel):
  - scores[t, c] = z_t . e_c - ||e_c||^2/2  computed on the PE with tokens on
    PSUM partitions and codes on the free dim.  The matmul runs in fp16
    hi/lo split form (z = zh + zl, e = e1 + e2; zh.e1 + zh.e2 + zl.e1) which
    carries ~fp32 precision at 3x the fp32 matmul rate; the per-code bias
    -||e||^2/2 is folded in as a 2-row fp16 matmul against a ones vector.
  - argmax over codes via DVE max8/find_index8 -> encoding index + best score
  - emb_q rows gathered from the DRAM codebook via indirect DMA, transposed
    back to channel-major with PE transpose, DMA'd out
  - sum(z^2) per chunk via ScalarE Square+accum (for the commitment loss)
Host: assembles shards and computes the scalar outputs (loss / perplexity /
usage) from the tiny per-core staging tensors.
"""

import os
import sys
from contextlib import ExitStack

import numpy as np

sys.path.insert(0, "/opt/trn_rl_repo")

import concourse.bacc as bacc
import concourse.bass as bass
import concourse.tile as tile
from concourse import mybir
from concourse.bass import IndirectOffsetOnAxis
from concourse.bass_utils import run_bass_kernel_spmd

F32 = mybir.dt.float32
F16 = mybir.dt.float16
U32 = mybir.dt.uint32

N_CORES = 8
B, D, T, H, W = 4, 256, 16, 32, 32
N_CODES = 2048
N_TOK = B * T * H * W            # 65536
S = N_TOK // N_CORES             # 8192 tokens per core
CHUNK = 512                      # tokens per load chunk
NCHUNK = S // CHUNK              # 16
MT = CHUNK // 128                # 4 token-tiles (128) per chunk
KT = D // 128                    # 2 contraction tiles
NC_CHUNK = 512                   # code chunk per matmul (psum bank)

MODE = os.environ.get("VQ_MODE", "f16x3")


def build_nc(nchunk=NCHUNK, mode=MODE):
    """Build the per-core Bass program. nchunk can be reduced for sim tests."""
    s = nchunk * CHUNK
    nmt = nchunk * MT
    nc = bacc.Bacc("TRN2", target_bir_lowering=False, debug=False)

    zt = nc.dram_tensor("zt", [KT, 128, s], F32, kind="ExternalInput")
    emb = nc.dram_tensor("emb", [N_CODES, D], F32, kind="ExternalInput")
    ident = nc.dram_tensor("ident", [128, 128], F32, kind="ExternalInput")
    if mode == "fp32":
        embT = nc.dram_tensor("embT", [KT, 128, N_CODES], F32,
                              kind="ExternalInput")
        biasr = nc.dram_tensor("biasr", [1, N_CODES], F32,
                               kind="ExternalInput")
    else:
        eh1 = nc.dram_tensor("eh1", [KT, 128, N_CODES], F16,
                             kind="ExternalInput")
        eh2 = nc.dram_tensor("eh2", [KT, 128, N_CODES], F16,
                             kind="ExternalInput")
        bias2 = nc.dram_tensor("bias2", [2, N_CODES], F16,
                               kind="ExternalInput")

    eq = nc.dram_tensor("eq", [KT, 128, s], F32, kind="ExternalOutput")
    idx_o = nc.dram_tensor("idx", [128, nmt], U32, kind="ExternalOutput")
    maxv_o = nc.dram_tensor("maxv", [128, nmt], F32, kind="ExternalOutput")
    sumsq_o = nc.dram_tensor("sumsq", [128, nchunk], F32,
                             kind="ExternalOutput")

    with tile.TileContext(nc) as tc, ExitStack() as ctx:
        consts = ctx.enter_context(tc.tile_pool(name="consts", bufs=1))
        zpool = ctx.enter_context(tc.tile_pool(name="z", bufs=3))
        zsplit = ctx.enter_context(tc.tile_pool(name="zs", bufs=3))
        scpool = ctx.enter_context(tc.tile_pool(name="scores", bufs=3))
        sqpool = ctx.enter_context(tc.tile_pool(name="sq", bufs=2))
        gpool = ctx.enter_context(tc.tile_pool(name="gather", bufs=3))
        opool = ctx.enter_context(tc.tile_pool(name="out", bufs=3))
        stage = ctx.enter_context(tc.tile_pool(name="stage", bufs=1))
        pspool = ctx.enter_context(tc.tile_pool(name="ps", bufs=3,
                                                space="PSUM"))
        tppool = ctx.enter_context(tc.tile_pool(name="tp", bufs=2,
                                                space="PSUM"))

        # --- constants resident in SBUF ---
        ident_sb = consts.tile([128, 128], F32)
        nc.sync.dma_start(out=ident_sb, in_=ident[:, :])
        if mode == "fp32":
            e_sb = consts.tile([128, KT, N_CODES], F32)
            for k in range(KT):
                nc.sync.dma_start(out=e_sb[:, k, :], in_=embT[k, :, :])
            biasr_sb = consts.tile([1, N_CODES], F32)
            nc.sync.dma_start(out=biasr_sb, in_=biasr[:, :])
            ones_sb = consts.tile([1, 128], F32)
            nc.vector.memset(ones_sb, 1.0)
        else:
            e1_sb = consts.tile([128, KT, N_CODES], F16)
            e2_sb = consts.tile([128, KT, N_CODES], F16)
            for k in range(KT):
                nc.sync.dma_start(out=e1_sb[:, k, :], in_=eh1[k, :, :])
                nc.sync.dma_start(out=e2_sb[:, k, :], in_=eh2[k, :, :])
            bias2_sb = consts.tile([2, N_CODES], F16)
            nc.sync.dma_start(out=bias2_sb, in_=bias2[:, :])
            ones2_sb = consts.tile([2, 128], F16)
            nc.vector.memset(ones2_sb, 1.0)

        # --- staging for small outputs ---
        idxstage = stage.tile([128, nmt, 8], U32)
        maxstage = stage.tile([128, nmt, 8], F32)
        sumsqstage = stage.tile([128, nchunk], F32)

        # PE warmup: dense dummy matmuls chew through the HAM cold window
        # while the big const/z DMAs land.
        warm = consts.tile([128, 128], F16)
        nc.vector.memset(warm, 0.0)
        wps = tppool.tile([128, 128], F32, tag="tp")
        for _ in range(40):
            nc.tensor.matmul(wps, lhsT=warm, rhs=warm, start=True, stop=True)

        for c in range(nchunk):
            t0 = c * CHUNK
            zc = zpool.tile([128, KT, CHUNK], F32)
            for k in range(KT):
                nc.sync.dma_start(out=zc[:, k, :], in_=zt[k, :, t0:t0 + CHUNK])

            # sum of squares of this chunk (both k-tiles at once)
            sq = sqpool.tile([128, KT, CHUNK], F32)
            nc.scalar.activation(
                out=sq, in_=zc,
                func=mybir.ActivationFunctionType.Square,
                accum_out=sumsqstage[:, c:c + 1],
            )

            if mode != "fp32":
                zh = zsplit.tile([128, KT, CHUNK], F16)
                nc.scalar.activation(
                    out=zh, in_=zc, func=mybir.ActivationFunctionType.Copy)
                zl = zsplit.tile([128, KT, CHUNK], F16)
                nc.gpsimd.tensor_tensor(
                    out=zl, in0=zc, in1=zh, op=mybir.AluOpType.subtract)

            for m in range(MT):
                mt = c * MT + m
                tl = m * 128

                sc = scpool.tile([128, N_CODES], F32)
                for half in range(2):
                    ps = pspool.tile([128, 2 * NC_CHUNK], F32)
                    for nl in range(2):
                        n = half * 2 + nl
                        pcols = slice(nl * NC_CHUNK, (nl + 1) * NC_CHUNK)
                        gcols = slice(n * NC_CHUNK, (n + 1) * NC_CHUNK)
                        if mode == "fp32":
                            for k in range(KT):
                                nc.tensor.matmul(
                                    ps[:, pcols],
                                    lhsT=zc[:, k, tl:tl + 128],
                                    rhs=e_sb[:, k, gcols],
                                    start=(k == 0), stop=False,
                                )
                            nc.tensor.matmul(
                                ps[:, pcols],
                                lhsT=ones_sb[:, :],
                                rhs=biasr_sb[:, gcols],
                                start=False, stop=True,
                            )
                        else:
                            first = True
                            for (zp, ep) in ((zh, e1_sb), (zh, e2_sb),
                                             (zl, e1_sb)):
                                for k in range(KT):
                                    nc.tensor.matmul(
                                        ps[:, pcols],
                                        lhsT=zp[:, k, tl:tl + 128],
                                        rhs=ep[:, k, gcols],
                                        start=first, stop=False,
                                    )
                                    first = False
                            nc.tensor.matmul(
                                ps[:, pcols],
                                lhsT=ones2_sb[:, :],
                                rhs=bias2_sb[:, gcols],
                                start=False, stop=True,
                            )
                    # PSUM -> SBUF so DVE max runs at SBUF speed, PSUM frees
                    nc.scalar.activation(
                        out=sc[:, half * 1024:(half + 1) * 1024], in_=ps,
                        func=mybir.ActivationFunctionType.Copy,
                    )

                nc.vector.max(out=maxstage[:, mt, :], in_=sc)
                nc.vector.max_index(
                    out=idxstage[:, mt, :],
                    in_max=maxstage[:, mt, :],
                    in_values=sc,
                )

                # gather emb rows for the 128 tokens of this tile
                eq_tok = gpool.tile([128, D], F32)
                nc.gpsimd.indirect_dma_start(
                    out=eq_tok,
                    out_offset=None,
                    in_=emb[:, :],
                    in_offset=IndirectOffsetOnAxis(
                        ap=idxstage[:, mt, 0:1], axis=0),
                )
                # transpose to channel-major and write out
                eqT = opool.tile([128, KT, 128], F32)
                for g in range(KT):
                    tp = tppool.tile([128, 128], F32)
                    nc.tensor.transpose(
                        tp, eq_tok[:, g * 128:(g + 1) * 128], ident_sb)
                    nc.scalar.activation(
                        out=eqT[:, g, :], in_=tp,
                        func=mybir.ActivationFunctionType.Copy,
                    )
                    nc.sync.dma_start(
                        out=eq[g, :, t0 + tl:t0 + tl + 128], in_=eqT[:, g, :])

                # flush idx/maxv stats every 16 token-tiles so the kernel
                # tail isn't serialized on one big compaction
                if mt % 16 == 15 or mt == nmt - 1:
                    f0 = (mt // 16) * 16
                    fw = mt - f0 + 1
                    idxf = opool.tile([128, 16], U32)
                    nc.vector.tensor_copy(
                        idxf[:, :fw], idxstage[:, f0:f0 + fw, 0])
                    maxf = opool.tile([128, 16], F32)
                    nc.vector.tensor_copy(
                        maxf[:, :fw], maxstage[:, f0:f0 + fw, 0])
                    nc.sync.dma_start(out=idx_o[:, f0:f0 + fw],
                                      in_=idxf[:, :fw])
                    nc.sync.dma_start(out=maxv_o[:, f0:f0 + fw],
                                      in_=maxf[:, :fw])

        nc.sync.dma_start(out=sumsq_o[:, :], in_=sumsqstage)

    nc.compile()
    return nc


_NC_CACHE = {}


def _get_nc():
    key = (NCHUNK, MODE)
    if key not in _NC_CACHE:
        _NC_CACHE[key] = build_nc()
    return _NC_CACHE[key]


def make_inputs(z, embeddings, mode=MODE):
    """Host-side prep: per-core input maps."""
    z = np.ascontiguousarray(np.asarray(z, dtype=np.float32))
    emb = np.ascontiguousarray(np.asarray(embeddings, dtype=np.float32))
    # [b, ktile, 128, s_total]
    zr = z.reshape(B, KT, 128, T * H * W)
    ident = np.eye(128, dtype=np.float32)
    common = {"emb": emb, "ident": ident}
    bias = (-0.5 * (emb.astype(np.float64) ** 2).sum(axis=1))
    if mode == "fp32":
        common["embT"] = np.ascontiguousarray(emb.T).reshape(KT, 128, N_CODES)
        common["biasr"] = bias.astype(np.float32)[None, :]
    else:
        embT = np.ascontiguousarray(emb.T).astype(np.float32)
        e1 = embT.astype(np.float16)
        e2 = (embT - e1.astype(np.float32)).astype(np.float16)
        common["eh1"] = e1.reshape(KT, 128, N_CODES)
        common["eh2"] = e2.reshape(KT, 128, N_CODES)
        bias32 = bias.astype(np.float32)
        b1 = bias32.astype(np.float16)
        b2 = (bias32 - b1.astype(np.float32)).astype(np.float16)
        common["bias2"] = np.stack([b1, b2])
    in_maps = []
    per_batch = T * H * W            # 16384
    cores_per_batch = per_batch // S  # 2
    for i in range(N_CORES):
        b = i // cores_per_batch
        off = (i % cores_per_batch) * S
        zt_i = np.ascontiguousarray(zr[b, :, :, off:off + S])
        in_maps.append({"zt": zt_i, **common})
    return in_maps


def postprocess(results, z=None):
    """Assemble full outputs from per-core result dicts."""
    eq_full = np.empty((B, D, T * H * W), dtype=np.float32)
    idx_full = np.empty((N_TOK,), dtype=np.int64)
    maxv_sum = 0.0
    sumsq_sum = 0.0
    per_batch = T * H * W
    cores_per_batch = per_batch // S
    for i, r in enumerate(results):
        b = i // cores_per_batch
        off = (i % cores_per_batch) * S
        eq_full[b, :, off:off + S] = r["eq"].reshape(D, S)
        # idx [128, nmt]: token (mt*128 + p) -> [p, mt]
        idx_full[i * S:(i + 1) * S] = r["idx"].T.reshape(S).astype(np.int64)
        maxv_sum += r["maxv"].astype(np.float64).sum()
        sumsq_sum += r["sumsq"].astype(np.float64).sum()

    embeddings_st = eq_full.reshape(B, D, T, H, W)
    encoding_indices = idx_full.reshape(B, T, H, W).astype(np.int32)

    # commitment loss. Device-side identity: sum (z - e*)^2 =
    # sum ||z||^2 - 2 * sum max_score (f64-accurate). To match the
    # reference's f32 arithmetic bit-for-bit, recompute the mean with the
    # same eager jax-CPU op sequence the reference uses when z is available.
    commitment_loss = None
    if z is not None and os.environ.get("VQ_FAST_LOSS", "0") != "1":
        try:
            import jax
            import jax.numpy as jnp
            with jax.default_device(jax.devices("cpu")[0]):
                zj = jnp.asarray(np.asarray(z, dtype=np.float32))
                qj = jnp.asarray(embeddings_st)
                cl = jnp.mean((zj - qj) ** 2)
                cl = jnp.minimum(cl, 10.0) * 0.25
                commitment_loss = np.float32(cl)
        except Exception:
            commitment_loss = None
    if commitment_loss is None:
        total = sumsq_sum - 2.0 * maxv_sum
        mean = np.float32(total / (N_TOK * D))
        commitment_loss = np.float32(min(float(mean), 10.0) * 0.25)

    counts = np.bincount(idx_full, minlength=N_CODES).astype(np.float32)
    avg_probs = counts / np.float32(N_TOK)
    perplexity = np.float32(
        np.exp(-np.sum(avg_probs * np.log(avg_probs + np.float32(1e-7)))))
    num_codes_used = np.int32((avg_probs > 0).sum())
    code_usage_ratio = np.float32(num_codes_used / np.float32(N_CODES))

    return (embeddings_st, encoding_indices, commitment_loss,
            perplexity, num_codes_used, code_usage_ratio)


def kernel(z, embeddings):
    nc = _get_nc()
    in_maps = make_inputs(z, embeddings)
    res = run_bass_kernel_spmd(nc, in_maps, core_ids=list(range(N_CORES)))
    return postprocess(res.results, z=z)


# revision 21
# speedup vs baseline: 1.0382x; 1.0344x over previous
"""VQ codebook eval forward on 8 Trainium2 NeuronCores.

Data-parallel over tokens: z [4, 256, 16, 32, 32] -> 65536 tokens of dim 256,
8192 tokens per core.  Codebook [2048, 256] replicated.

Per core (Bass/Tile kernel):
  - scores[t, c] = z_t . e_c - ||e_c||^2/2  computed on the PE with tokens on
    PSUM partitions and codes on the free dim.  The matmul runs in fp16
    hi/lo split form (z = zh + zl, e = e1 + e2; zh.e1 + zh.e2 + zl.e1) which
    carries ~fp32 precision at 3x the fp32 matmul rate; the per-code bias
    -||e||^2/2 is folded in as a 2-row fp16 matmul against a ones vector.
  - argmax over codes via DVE max8/find_index8 -> encoding index + best score
  - emb_q rows gathered from the DRAM codebook via indirect DMA, transposed
    back to channel-major with PE transpose, DMA'd out
  - sum(z^2) per chunk via ScalarE Square+accum (for the commitment loss)
Host: assembles shards and computes the scalar outputs (loss / perplexity /
usage) from the tiny per-core staging tensors.
"""

import os
import sys
from contextlib import ExitStack

import numpy as np

sys.path.insert(0, "/opt/trn_rl_repo")

import concourse.bacc as bacc
import concourse.bass as bass
import concourse.tile as tile
from concourse import mybir
from concourse.bass import IndirectOffsetOnAxis
from concourse.bass_utils import run_bass_kernel_spmd

F32 = mybir.dt.float32
F16 = mybir.dt.float16
U32 = mybir.dt.uint32

N_CORES = 8
B, D, T, H, W = 4, 256, 16, 32, 32
N_CODES = 2048
N_TOK = B * T * H * W            # 65536
S = N_TOK // N_CORES             # 8192 tokens per core
CHUNK = 512                      # tokens per load chunk
NCHUNK = S // CHUNK              # 16
MT = CHUNK // 128                # 4 token-tiles (128) per chunk
KT = D // 128                    # 2 contraction tiles
NC_CHUNK = 512                   # code chunk per matmul (psum bank)

MODE = os.environ.get("VQ_MODE", "f16x3")


def build_nc(nchunk=NCHUNK, mode=MODE):
    """Build the per-core Bass program. nchunk can be reduced for sim tests."""
    s = nchunk * CHUNK
    nmt = nchunk * MT
    nc = bacc.Bacc("TRN2", target_bir_lowering=False, debug=False)

    zt = nc.dram_tensor("zt", [KT, 128, s], F32, kind="ExternalInput")
    emb = nc.dram_tensor("emb", [N_CODES, D], F32, kind="ExternalInput")
    ident = nc.dram_tensor("ident", [128, 128], F32, kind="ExternalInput")
    if mode == "fp32":
        embT = nc.dram_tensor("embT", [KT, 128, N_CODES], F32,
                              kind="ExternalInput")
        biasr = nc.dram_tensor("biasr", [1, N_CODES], F32,
                               kind="ExternalInput")
    else:
        eh1 = nc.dram_tensor("eh1", [KT, 128, N_CODES], F16,
                             kind="ExternalInput")
        eh2 = nc.dram_tensor("eh2", [KT, 128, N_CODES], F16,
                             kind="ExternalInput")
        bias2 = nc.dram_tensor("bias2", [2, N_CODES], F16,
                               kind="ExternalInput")

    eq = nc.dram_tensor("eq", [KT, 128, s], F32, kind="ExternalOutput")
    idx_o = nc.dram_tensor("idx", [128, nmt], U32, kind="ExternalOutput")
    maxv_o = nc.dram_tensor("maxv", [128, nmt], F32, kind="ExternalOutput")
    sumsq_o = nc.dram_tensor("sumsq", [128, nchunk], F32,
                             kind="ExternalOutput")

    with tile.TileContext(nc) as tc, ExitStack() as ctx:
        consts = ctx.enter_context(tc.tile_pool(name="consts", bufs=1))
        zpool = ctx.enter_context(tc.tile_pool(name="z", bufs=4))
        zsplit = ctx.enter_context(tc.tile_pool(name="zs", bufs=3))
        scpool = ctx.enter_context(tc.tile_pool(name="scores", bufs=3))
        sqpool = ctx.enter_context(tc.tile_pool(name="sq", bufs=2))
        gpool = ctx.enter_context(tc.tile_pool(name="gather", bufs=3))
        opool = ctx.enter_context(tc.tile_pool(name="out", bufs=3))
        stage = ctx.enter_context(tc.tile_pool(name="stage", bufs=1))
        pspool = ctx.enter_context(tc.tile_pool(name="ps", bufs=3,
                                                space="PSUM"))
        tppool = ctx.enter_context(tc.tile_pool(name="tp", bufs=2,
                                                space="PSUM"))

        # --- constants resident in SBUF ---
        ident_sb = consts.tile([128, 128], F32)
        nc.sync.dma_start(out=ident_sb, in_=ident[:, :])
        if mode == "fp32":
            e_sb = consts.tile([128, KT, N_CODES], F32)
            for k in range(KT):
                nc.sync.dma_start(out=e_sb[:, k, :], in_=embT[k, :, :])
            biasr_sb = consts.tile([1, N_CODES], F32)
            nc.sync.dma_start(out=biasr_sb, in_=biasr[:, :])
            ones_sb = consts.tile([1, 128], F32)
            nc.vector.memset(ones_sb, 1.0)
        else:
            e1_sb = consts.tile([128, KT, N_CODES], F16)
            e2_sb = consts.tile([128, KT, N_CODES], F16)
            for k in range(KT):
                nc.sync.dma_start(out=e1_sb[:, k, :], in_=eh1[k, :, :])
                nc.sync.dma_start(out=e2_sb[:, k, :], in_=eh2[k, :, :])
            bias2_sb = consts.tile([2, N_CODES], F16)
            nc.sync.dma_start(out=bias2_sb, in_=bias2[:, :])
            ones2_sb = consts.tile([2, 128], F16)
            nc.vector.memset(ones2_sb, 1.0)

        # --- staging for small outputs ---
        idxstage = stage.tile([128, nmt, 8], U32)
        maxstage = stage.tile([128, nmt, 8], F32)
        sumsqstage = stage.tile([128, nchunk], F32)

        # PE warmup: dense dummy matmuls chew through the HAM cold window
        # while the big const/z DMAs land.
        warm = consts.tile([128, 128], F16)
        nc.vector.memset(warm, 0.0)
        wps = tppool.tile([128, 128], F32, tag="tp")
        for _ in range(96):
            nc.tensor.matmul(wps, lhsT=warm, rhs=warm, start=True, stop=True)

        for c in range(nchunk):
            t0 = c * CHUNK
            zc = zpool.tile([128, KT, CHUNK], F32)
            for k in range(KT):
                nc.sync.dma_start(out=zc[:, k, :], in_=zt[k, :, t0:t0 + CHUNK])

            # sum of squares of this chunk (both k-tiles at once)
            sq = sqpool.tile([128, KT, CHUNK], F32)
            nc.scalar.activation(
                out=sq, in_=zc,
                func=mybir.ActivationFunctionType.Square,
                accum_out=sumsqstage[:, c:c + 1],
            )

            if mode != "fp32":
                zh = zsplit.tile([128, KT, CHUNK], F16)
                nc.scalar.activation(
                    out=zh, in_=zc, func=mybir.ActivationFunctionType.Copy)
                zl = zsplit.tile([128, KT, CHUNK], F16)
                nc.gpsimd.tensor_tensor(
                    out=zl, in0=zc, in1=zh, op=mybir.AluOpType.subtract)

            for m in range(MT):
                mt = c * MT + m
                tl = m * 128

                sc = scpool.tile([128, N_CODES], F32)
                for half in range(2):
                    ps = pspool.tile([128, 2 * NC_CHUNK], F32)
                    for nl in range(2):
                        n = half * 2 + nl
                        pcols = slice(nl * NC_CHUNK, (nl + 1) * NC_CHUNK)
                        gcols = slice(n * NC_CHUNK, (n + 1) * NC_CHUNK)
                        if mode == "fp32":
                            for k in range(KT):
                                nc.tensor.matmul(
                                    ps[:, pcols],
                                    lhsT=zc[:, k, tl:tl + 128],
                                    rhs=e_sb[:, k, gcols],
                                    start=(k == 0), stop=False,
                                )
                            nc.tensor.matmul(
                                ps[:, pcols],
                                lhsT=ones_sb[:, :],
                                rhs=biasr_sb[:, gcols],
                                start=False, stop=True,
                            )
                        else:
                            first = True
                            for (zp, ep) in ((zh, e1_sb), (zh, e2_sb),
                                             (zl, e1_sb)):
                                for k in range(KT):
                                    nc.tensor.matmul(
                                        ps[:, pcols],
                                        lhsT=zp[:, k, tl:tl + 128],
                                        rhs=ep[:, k, gcols],
                                        start=first, stop=False,
                                    )
                                    first = False
                            nc.tensor.matmul(
                                ps[:, pcols],
                                lhsT=ones2_sb[:, :],
                                rhs=bias2_sb[:, gcols],
                                start=False, stop=True,
                            )
                    # PSUM -> SBUF so DVE max runs at SBUF speed, PSUM frees
                    nc.scalar.activation(
                        out=sc[:, half * 1024:(half + 1) * 1024], in_=ps,
                        func=mybir.ActivationFunctionType.Copy,
                    )

                nc.vector.max(out=maxstage[:, mt, :], in_=sc)
                nc.vector.max_index(
                    out=idxstage[:, mt, :],
                    in_max=maxstage[:, mt, :],
                    in_values=sc,
                )

                # gather emb rows for the 128 tokens of this tile
                eq_tok = gpool.tile([128, D], F32)
                nc.gpsimd.indirect_dma_start(
                    out=eq_tok,
                    out_offset=None,
                    in_=emb[:, :],
                    in_offset=IndirectOffsetOnAxis(
                        ap=idxstage[:, mt, 0:1], axis=0),
                )
                # transpose to channel-major and write out
                eqT = opool.tile([128, KT, 128], F32)
                for g in range(KT):
                    tp = tppool.tile([128, 128], F32)
                    nc.tensor.transpose(
                        tp, eq_tok[:, g * 128:(g + 1) * 128], ident_sb)
                    nc.scalar.activation(
                        out=eqT[:, g, :], in_=tp,
                        func=mybir.ActivationFunctionType.Copy,
                    )
                    nc.sync.dma_start(
                        out=eq[g, :, t0 + tl:t0 + tl + 128], in_=eqT[:, g, :])

                # flush idx/maxv stats every 16 token-tiles so the kernel
                # tail isn't serialized on one big compaction
                if mt % 16 == 15 or mt == nmt - 1:
                    f0 = (mt // 16) * 16
                    fw = mt - f0 + 1
                    idxf = opool.tile([128, 16], U32)
                    nc.vector.tensor_copy(
                        idxf[:, :fw], idxstage[:, f0:f0 + fw, 0])
                    maxf = opool.tile([128, 16], F32)
                    nc.vector.tensor_copy(
                        maxf[:, :fw], maxstage[:, f0:f0 + fw, 0])
                    nc.sync.dma_start(out=idx_o[:, f0:f0 + fw],
                                      in_=idxf[:, :fw])
                    nc.sync.dma_start(out=maxv_o[:, f0:f0 + fw],
                                      in_=maxf[:, :fw])

        nc.sync.dma_start(out=sumsq_o[:, :], in_=sumsqstage)

    nc.compile()
    return nc


_NC_CACHE = {}


def _get_nc():
    key = (NCHUNK, MODE)
    if key not in _NC_CACHE:
        _NC_CACHE[key] = build_nc()
    return _NC_CACHE[key]


def make_inputs(z, embeddings, mode=MODE):
    """Host-side prep: per-core input maps."""
    z = np.ascontiguousarray(np.asarray(z, dtype=np.float32))
    emb = np.ascontiguousarray(np.asarray(embeddings, dtype=np.float32))
    # [b, ktile, 128, s_total]
    zr = z.reshape(B, KT, 128, T * H * W)
    ident = np.eye(128, dtype=np.float32)
    common = {"emb": emb, "ident": ident}
    bias = (-0.5 * (emb.astype(np.float64) ** 2).sum(axis=1))
    if mode == "fp32":
        common["embT"] = np.ascontiguousarray(emb.T).reshape(KT, 128, N_CODES)
        common["biasr"] = bias.astype(np.float32)[None, :]
    else:
        embT = np.ascontiguousarray(emb.T).astype(np.float32)
        e1 = embT.astype(np.float16)
        e2 = (embT - e1.astype(np.float32)).astype(np.float16)
        common["eh1"] = e1.reshape(KT, 128, N_CODES)
        common["eh2"] = e2.reshape(KT, 128, N_CODES)
        bias32 = bias.astype(np.float32)
        b1 = bias32.astype(np.float16)
        b2 = (bias32 - b1.astype(np.float32)).astype(np.float16)
        common["bias2"] = np.stack([b1, b2])
    in_maps = []
    per_batch = T * H * W            # 16384
    cores_per_batch = per_batch // S  # 2
    for i in range(N_CORES):
        b = i // cores_per_batch
        off = (i % cores_per_batch) * S
        zt_i = np.ascontiguousarray(zr[b, :, :, off:off + S])
        in_maps.append({"zt": zt_i, **common})
    return in_maps


def postprocess(results, z=None):
    """Assemble full outputs from per-core result dicts."""
    eq_full = np.empty((B, D, T * H * W), dtype=np.float32)
    idx_full = np.empty((N_TOK,), dtype=np.int64)
    maxv_sum = 0.0
    sumsq_sum = 0.0
    per_batch = T * H * W
    cores_per_batch = per_batch // S
    for i, r in enumerate(results):
        b = i // cores_per_batch
        off = (i % cores_per_batch) * S
        eq_full[b, :, off:off + S] = r["eq"].reshape(D, S)
        # idx [128, nmt]: token (mt*128 + p) -> [p, mt]
        idx_full[i * S:(i + 1) * S] = r["idx"].T.reshape(S).astype(np.int64)
        maxv_sum += r["maxv"].astype(np.float64).sum()
        sumsq_sum += r["sumsq"].astype(np.float64).sum()

    embeddings_st = eq_full.reshape(B, D, T, H, W)
    encoding_indices = idx_full.reshape(B, T, H, W).astype(np.int32)

    # commitment loss. Device-side identity: sum (z - e*)^2 =
    # sum ||z||^2 - 2 * sum max_score (f64-accurate). To match the
    # reference's f32 arithmetic bit-for-bit, recompute the mean with the
    # same eager jax-CPU op sequence the reference uses when z is available.
    commitment_loss = None
    if z is not None and os.environ.get("VQ_FAST_LOSS", "0") != "1":
        try:
            import jax
            import jax.numpy as jnp
            with jax.default_device(jax.devices("cpu")[0]):
                zj = jnp.asarray(np.asarray(z, dtype=np.float32))
                qj = jnp.asarray(embeddings_st)
                cl = jnp.mean((zj - qj) ** 2)
                cl = jnp.minimum(cl, 10.0) * 0.25
                commitment_loss = np.float32(cl)
        except Exception:
            commitment_loss = None
    if commitment_loss is None:
        total = sumsq_sum - 2.0 * maxv_sum
        mean = np.float32(total / (N_TOK * D))
        commitment_loss = np.float32(min(float(mean), 10.0) * 0.25)

    counts = np.bincount(idx_full, minlength=N_CODES).astype(np.float32)
    avg_probs = counts / np.float32(N_TOK)
    perplexity = np.float32(
        np.exp(-np.sum(avg_probs * np.log(avg_probs + np.float32(1e-7)))))
    num_codes_used = np.int32((avg_probs > 0).sum())
    code_usage_ratio = np.float32(num_codes_used / np.float32(N_CODES))

    return (embeddings_st, encoding_indices, commitment_loss,
            perplexity, num_codes_used, code_usage_ratio)


def kernel(z, embeddings):
    nc = _get_nc()
    in_maps = make_inputs(z, embeddings)
    res = run_bass_kernel_spmd(nc, in_maps, core_ids=list(range(N_CORES)))
    return postprocess(res.results, z=z)


# revision 27
# speedup vs baseline: 1.2481x; 1.2021x over previous
"""VQ codebook eval forward on 8 Trainium2 NeuronCores.

Data-parallel over tokens: z [4, 256, 16, 32, 32] -> 65536 tokens of dim 256,
8192 tokens per core.  Codebook [2048, 256] replicated.

Per core (Bass/Tile kernel):
  - scores[t, c] = z_t . e_c - ||e_c||^2/2  computed on the PE with tokens on
    PSUM partitions and codes on the free dim.  The matmul runs in fp16
    hi/lo split form (z = zh + zl, e = e1 + e2; zh.e1 + zh.e2 + zl.e1) which
    carries ~fp32 precision at 3x the fp32 matmul rate; the per-code bias
    -||e||^2/2 is folded in as a 2-row fp16 matmul against a ones vector.
  - argmax over codes via DVE max8/find_index8 -> encoding index + best score
  - emb_q rows gathered from the DRAM codebook via indirect DMA, transposed
    back to channel-major with PE transpose, DMA'd out
  - sum(z^2) per chunk via ScalarE Square+accum (for the commitment loss)
Host: assembles shards and computes the scalar outputs (loss / perplexity /
usage) from the tiny per-core staging tensors.
"""

import os
import sys
from contextlib import ExitStack

import numpy as np

sys.path.insert(0, "/opt/trn_rl_repo")

import concourse.bacc as bacc
import concourse.bass as bass
import concourse.tile as tile
from concourse import mybir
from concourse.bass import IndirectOffsetOnAxis
from concourse.bass_utils import run_bass_kernel_spmd

F32 = mybir.dt.float32
F16 = mybir.dt.float16
U32 = mybir.dt.uint32

N_CORES = 8
B, D, T, H, W = 4, 256, 16, 32, 32
N_CODES = 2048
N_TOK = B * T * H * W            # 65536
S = N_TOK // N_CORES             # 8192 tokens per core
CHUNK = 512                      # tokens per load chunk
NCHUNK = S // CHUNK              # 16
MT = CHUNK // 128                # 4 token-tiles (128) per chunk
KT = D // 128                    # 2 contraction tiles
NC_CHUNK = 512                   # code chunk per matmul (psum bank)

MODE = os.environ.get("VQ_MODE", "f16x3")


def build_nc(nchunk=NCHUNK, mode=MODE):
    """Build the per-core Bass program. nchunk can be reduced for sim tests."""
    s = nchunk * CHUNK
    nmt = nchunk * MT
    nc = bacc.Bacc("TRN2", target_bir_lowering=False, debug=False)

    zt = nc.dram_tensor("zt", [KT, 128, s], F32, kind="ExternalInput")
    emb = nc.dram_tensor("emb", [N_CODES, D], F32, kind="ExternalInput")
    ident = nc.dram_tensor("ident", [128, 128], F32, kind="ExternalInput")
    if mode == "fp32":
        embT = nc.dram_tensor("embT", [KT, 128, N_CODES], F32,
                              kind="ExternalInput")
        biasr = nc.dram_tensor("biasr", [1, N_CODES], F32,
                               kind="ExternalInput")
    else:
        eh1 = nc.dram_tensor("eh1", [KT, 128, N_CODES], F16,
                             kind="ExternalInput")
        eh2 = nc.dram_tensor("eh2", [KT, 128, N_CODES], F16,
                             kind="ExternalInput")
        biasf = nc.dram_tensor("biasf", [1, N_CODES], F32,
                               kind="ExternalInput")

    eq = nc.dram_tensor("eq", [KT, 128, s], F32, kind="ExternalOutput")
    idx_o = nc.dram_tensor("idx", [128, nmt], U32, kind="ExternalOutput")
    maxv_o = nc.dram_tensor("maxv", [128, nmt], F32, kind="ExternalOutput")
    sumsq_o = nc.dram_tensor("sumsq", [128, nchunk], F32,
                             kind="ExternalOutput")

    with tile.TileContext(nc) as tc, ExitStack() as ctx:
        consts = ctx.enter_context(tc.tile_pool(name="consts", bufs=1))
        zpool = ctx.enter_context(tc.tile_pool(name="z", bufs=4))
        zsplit = ctx.enter_context(tc.tile_pool(name="zs", bufs=3))
        scpool = ctx.enter_context(tc.tile_pool(name="scores", bufs=3))
        sqpool = ctx.enter_context(tc.tile_pool(name="sq", bufs=2))
        gpool = ctx.enter_context(tc.tile_pool(name="gather", bufs=3))
        opool = ctx.enter_context(tc.tile_pool(name="out", bufs=3))
        stage = ctx.enter_context(tc.tile_pool(name="stage", bufs=1))
        pspool = ctx.enter_context(tc.tile_pool(name="ps", bufs=3,
                                                space="PSUM"))
        tppool = ctx.enter_context(tc.tile_pool(name="tp", bufs=2,
                                                space="PSUM"))

        # --- constants resident in SBUF ---
        ident_sb = consts.tile([128, 128], F32)
        nc.sync.dma_start(out=ident_sb, in_=ident[:, :])
        if mode == "fp32":
            e_sb = consts.tile([128, KT, N_CODES], F32)
            for k in range(KT):
                nc.sync.dma_start(out=e_sb[:, k, :], in_=embT[k, :, :])
            biasr_sb = consts.tile([1, N_CODES], F32)
            nc.sync.dma_start(out=biasr_sb, in_=biasr[:, :])
            ones_sb = consts.tile([1, 128], F32)
            nc.vector.memset(ones_sb, 1.0)
        else:
            e1_sb = consts.tile([128, KT, N_CODES], F16)
            e2_sb = consts.tile([128, KT, N_CODES], F16)
            for k in range(KT):
                nc.sync.dma_start(out=e1_sb[:, k, :], in_=eh1[k, :, :])
                nc.sync.dma_start(out=e2_sb[:, k, :], in_=eh2[k, :, :])
            # bias row broadcast to all 128 partitions (stride-0 DMA) so a
            # ScalarE copy can preload it into PSUM before each matmul group
            bias_bcast = consts.tile([128, N_CODES], F32)
            bap = biasf[:, :]
            bcast_ap = bass.AP(tensor=bap.tensor, offset=bap.offset,
                               ap=[[0, 128], bap.ap[1]])
            nc.gpsimd.dma_start(out=bias_bcast, in_=bcast_ap)

        # --- staging for small outputs ---
        idxstage = stage.tile([128, nmt, 8], U32)
        maxstage = stage.tile([128, nmt, 8], F32)
        sumsqstage = stage.tile([128, nchunk], F32)

        # PE warmup: dense dummy matmuls chew through the HAM cold window
        # while the big const/z DMAs land.
        warm = consts.tile([128, 128], F16)
        nc.vector.memset(warm, 0.0)
        wps = tppool.tile([128, 128], F32, tag="tp")
        for _ in range(96):
            nc.tensor.matmul(wps, lhsT=warm, rhs=warm, start=True, stop=True)

        if mode != "fp32":
            # Prime the score-PSUM buffers: one start=True (never stopped)
            # zero-matmul per bank sets every has_written bit, so all later
            # groups can accumulate (start=False) on top of values a ScalarE
            # copy preloads into PSUM. The 3 concurrently-live tiles pin the
            # pool's 3 physical slots.
            warm512 = consts.tile([128, 512], F16)
            nc.vector.memset(warm512, 0.0)
            for _ in range(3):
                ptile = pspool.tile([128, 2 * NC_CHUNK], F32, tag="ps")
                for h2 in range(2):
                    nc.tensor.matmul(
                        ptile[:, h2 * 512:(h2 + 1) * 512],
                        lhsT=warm, rhs=warm512,
                        start=True, stop=False, skip_group_check=True)

        for c in range(nchunk):
            t0 = c * CHUNK
            zc = zpool.tile([128, KT, CHUNK], F32)
            for k in range(KT):
                nc.sync.dma_start(out=zc[:, k, :], in_=zt[k, :, t0:t0 + CHUNK])

            # sum of squares of this chunk (both k-tiles at once)
            sq = sqpool.tile([128, KT, CHUNK], F32)
            nc.scalar.activation(
                out=sq, in_=zc,
                func=mybir.ActivationFunctionType.Square,
                accum_out=sumsqstage[:, c:c + 1],
            )

            if mode != "fp32":
                zh = zsplit.tile([128, KT, CHUNK], F16)
                nc.scalar.activation(
                    out=zh, in_=zc, func=mybir.ActivationFunctionType.Copy)
                zl = zsplit.tile([128, KT, CHUNK], F16)
                nc.gpsimd.tensor_tensor(
                    out=zl, in0=zc, in1=zh, op=mybir.AluOpType.subtract)

            for m in range(MT):
                mt = c * MT + m
                tl = m * 128

                sc = scpool.tile([128, N_CODES], F32)
                for half in range(2):
                    ps = pspool.tile([128, 2 * NC_CHUNK], F32, tag="ps")
                    if mode != "fp32":
                        # preload bias into PSUM; matmuls accumulate onto it
                        nc.scalar.activation(
                            out=ps,
                            in_=bias_bcast[:, half * 1024:(half + 1) * 1024],
                            func=mybir.ActivationFunctionType.Copy,
                        )
                    for nl in range(2):
                        n = half * 2 + nl
                        pcols = slice(nl * NC_CHUNK, (nl + 1) * NC_CHUNK)
                        gcols = slice(n * NC_CHUNK, (n + 1) * NC_CHUNK)
                        if mode == "fp32":
                            for k in range(KT):
                                nc.tensor.matmul(
                                    ps[:, pcols],
                                    lhsT=zc[:, k, tl:tl + 128],
                                    rhs=e_sb[:, k, gcols],
                                    start=(k == 0), stop=False,
                                )
                            nc.tensor.matmul(
                                ps[:, pcols],
                                lhsT=ones_sb[:, :],
                                rhs=biasr_sb[:, gcols],
                                start=False, stop=True,
                            )
                        else:
                            for (zp, ep) in ((zh, e1_sb), (zh, e2_sb),
                                             (zl, e1_sb)):
                                for k in range(KT):
                                    nc.tensor.matmul(
                                        ps[:, pcols],
                                        lhsT=zp[:, k, tl:tl + 128],
                                        rhs=ep[:, k, gcols],
                                        start=False, stop=False,
                                        skip_group_check=True,
                                    )
                    # PSUM -> SBUF so DVE max runs at SBUF speed, PSUM frees
                    nc.scalar.activation(
                        out=sc[:, half * 1024:(half + 1) * 1024], in_=ps,
                        func=mybir.ActivationFunctionType.Copy,
                    )

                nc.vector.max(out=maxstage[:, mt, :], in_=sc)
                nc.vector.max_index(
                    out=idxstage[:, mt, :],
                    in_max=maxstage[:, mt, :],
                    in_values=sc,
                )

                # gather emb rows for the 128 tokens of this tile
                eq_tok = gpool.tile([128, D], F32)
                nc.gpsimd.indirect_dma_start(
                    out=eq_tok,
                    out_offset=None,
                    in_=emb[:, :],
                    in_offset=IndirectOffsetOnAxis(
                        ap=idxstage[:, mt, 0:1], axis=0),
                )
                # transpose to channel-major and write out
                eqT = opool.tile([128, KT, 128], F32)
                for g in range(KT):
                    tp = tppool.tile([128, 128], F32)
                    nc.tensor.transpose(
                        tp, eq_tok[:, g * 128:(g + 1) * 128], ident_sb)
                    nc.scalar.activation(
                        out=eqT[:, g, :], in_=tp,
                        func=mybir.ActivationFunctionType.Copy,
                    )
                    nc.sync.dma_start(
                        out=eq[g, :, t0 + tl:t0 + tl + 128], in_=eqT[:, g, :])

                # flush idx/maxv stats every 16 token-tiles so the kernel
                # tail isn't serialized on one big compaction
                if mt % 16 == 15 or mt == nmt - 1:
                    f0 = (mt // 16) * 16
                    fw = mt - f0 + 1
                    idxf = opool.tile([128, 16], U32)
                    nc.vector.tensor_copy(
                        idxf[:, :fw], idxstage[:, f0:f0 + fw, 0])
                    maxf = opool.tile([128, 16], F32)
                    nc.vector.tensor_copy(
                        maxf[:, :fw], maxstage[:, f0:f0 + fw, 0])
                    nc.sync.dma_start(out=idx_o[:, f0:f0 + fw],
                                      in_=idxf[:, :fw])
                    nc.sync.dma_start(out=maxv_o[:, f0:f0 + fw],
                                      in_=maxf[:, :fw])

        nc.sync.dma_start(out=sumsq_o[:, :], in_=sumsqstage)

    nc.compile()
    return nc


_NC_CACHE = {}


def _get_nc():
    key = (NCHUNK, MODE)
    if key not in _NC_CACHE:
        _NC_CACHE[key] = build_nc()
    return _NC_CACHE[key]


def make_inputs(z, embeddings, mode=MODE):
    """Host-side prep: per-core input maps."""
    z = np.ascontiguousarray(np.asarray(z, dtype=np.float32))
    emb = np.ascontiguousarray(np.asarray(embeddings, dtype=np.float32))
    # [b, ktile, 128, s_total]
    zr = z.reshape(B, KT, 128, T * H * W)
    ident = np.eye(128, dtype=np.float32)
    common = {"emb": emb, "ident": ident}
    bias = (-0.5 * (emb.astype(np.float64) ** 2).sum(axis=1))
    if mode == "fp32":
        common["embT"] = np.ascontiguousarray(emb.T).reshape(KT, 128, N_CODES)
        common["biasr"] = bias.astype(np.float32)[None, :]
    else:
        embT = np.ascontiguousarray(emb.T).astype(np.float32)
        e1 = embT.astype(np.float16)
        e2 = (embT - e1.astype(np.float32)).astype(np.float16)
        common["eh1"] = e1.reshape(KT, 128, N_CODES)
        common["eh2"] = e2.reshape(KT, 128, N_CODES)
        common["biasf"] = bias.astype(np.float32)[None, :]
    in_maps = []
    per_batch = T * H * W            # 16384
    cores_per_batch = per_batch // S  # 2
    for i in range(N_CORES):
        b = i // cores_per_batch
        off = (i % cores_per_batch) * S
        zt_i = np.ascontiguousarray(zr[b, :, :, off:off + S])
        in_maps.append({"zt": zt_i, **common})
    return in_maps


def postprocess(results, z=None):
    """Assemble full outputs from per-core result dicts."""
    eq_full = np.empty((B, D, T * H * W), dtype=np.float32)
    idx_full = np.empty((N_TOK,), dtype=np.int64)
    maxv_sum = 0.0
    sumsq_sum = 0.0
    per_batch = T * H * W
    cores_per_batch = per_batch // S
    for i, r in enumerate(results):
        b = i // cores_per_batch
        off = (i % cores_per_batch) * S
        eq_full[b, :, off:off + S] = r["eq"].reshape(D, S)
        # idx [128, nmt]: token (mt*128 + p) -> [p, mt]
        idx_full[i * S:(i + 1) * S] = r["idx"].T.reshape(S).astype(np.int64)
        maxv_sum += r["maxv"].astype(np.float64).sum()
        sumsq_sum += r["sumsq"].astype(np.float64).sum()

    embeddings_st = eq_full.reshape(B, D, T, H, W)
    encoding_indices = idx_full.reshape(B, T, H, W).astype(np.int32)

    # commitment loss. Device-side identity: sum (z - e*)^2 =
    # sum ||z||^2 - 2 * sum max_score (f64-accurate). To match the
    # reference's f32 arithmetic bit-for-bit, recompute the mean with the
    # same eager jax-CPU op sequence the reference uses when z is available.
    commitment_loss = None
    if z is not None and os.environ.get("VQ_FAST_LOSS", "0") != "1":
        try:
            import jax
            import jax.numpy as jnp
            with jax.default_device(jax.devices("cpu")[0]):
                zj = jnp.asarray(np.asarray(z, dtype=np.float32))
                qj = jnp.asarray(embeddings_st)
                cl = jnp.mean((zj - qj) ** 2)
                cl = jnp.minimum(cl, 10.0) * 0.25
                commitment_loss = np.float32(cl)
        except Exception:
            commitment_loss = None
    if commitment_loss is None:
        total = sumsq_sum - 2.0 * maxv_sum
        mean = np.float32(total / (N_TOK * D))
        commitment_loss = np.float32(min(float(mean), 10.0) * 0.25)

    counts = np.bincount(idx_full, minlength=N_CODES).astype(np.float32)
    avg_probs = counts / np.float32(N_TOK)
    perplexity = np.float32(
        np.exp(-np.sum(avg_probs * np.log(avg_probs + np.float32(1e-7)))))
    num_codes_used = np.int32((avg_probs > 0).sum())
    code_usage_ratio = np.float32(num_codes_used / np.float32(N_CODES))

    return (embeddings_st, encoding_indices, commitment_loss,
            perplexity, num_codes_used, code_usage_ratio)


def kernel(z, embeddings):
    nc = _get_nc()
    in_maps = make_inputs(z, embeddings)
    res = run_bass_kernel_spmd(nc, in_maps, core_ids=list(range(N_CORES)))
    return postprocess(res.results, z=z)
